# revision 94
# baseline (speedup 1.0000x reference)
"""Trainium2 Bass kernel for CelltypeDeconvolver (GCN message passing).

Runs SPMD on 8 NeuronCores. Nodes are partitioned across cores; per GCN
layer each core computes h_pre = H @ W for its nodes (scaled by
dinv[src]), AllGathers the scaled features into a replicated DRAM table,
then dma_gathers edge source rows (two streams: low/high table half,
chunked across dst windows) and segment-reduces them on the TensorEngine
with host-built 0/1 fp8 selection matrices into per-window PSUM tiles.
Self-loop terms are applied locally in the epilogue. Graph structure
(edge bucketing, degrees, padding, selection matrices) is prepared
host-side in numpy; all float math on node features happens on-device.
"""

import contextlib
import ctypes
import os
import sys
import types

import numpy as np

for _p in ("/opt/trn_rl_repo",):
    if os.path.isdir(_p) and _p not in sys.path:
        sys.path.append(_p)

import ml_dtypes

import concourse.bass as bass
import concourse.bacc as bacc
import concourse.mybir as mybir
from concourse import library_config
from concourse.tile import TileContext
from concourse.bass_utils import run_bass_kernel_spmd

BF16 = mybir.dt.bfloat16
F32 = mybir.dt.float32
FP8 = mybir.dt.float8e4
I16 = mybir.dt.int16
AX = mybir.AluOpType
AFT = mybir.ActivationFunctionType
NP8 = mybir.dt.np(FP8)

NCORES = 8
P = 128
BN_EPS = 1e-5
CB = 16            # gather/S chunk size in 128-slot blocks

TRACE = False
TRACE_KW = {}
LAST = {}
_CACHE = {}
ASYNC_GATHER = False
NAG = 2            # AllGather chunks per layer
AG_SPLIT = 0.51    # fraction of windows in the first (overlapped) AG chunk
FP8_TBL = False    # feature table / gathered rows in fp8e4m3
BALANCE = True     # permute nodes to balance per-window edge buckets


def _pack_k(w, kpad):
    """[K, N] f32 -> [K2, 128, 2, N] bf16 packed (k = k2*256 + r*128 + p)."""
    w = np.asarray(w, np.float32)
    k, n = w.shape
    wp = np.zeros((kpad, n), np.float32)
    wp[:k] = w
    k2 = kpad // 256
    return np.ascontiguousarray(
        wp.reshape(k2, 2, P, n).transpose(0, 2, 1, 3)).astype(ml_dtypes.bfloat16)


def _cdiv(a, b):
    return (a + b - 1) // b


def _ag_bounds(W):
    if NAG == 2:
        return [max(1, min(W - 1, int(round(W * AG_SPLIT)))), W]
    return [W * (k + 1) // NAG for k in range(NAG)]


# ──────────────────────────────────────────────────────────────────────
# host-side plan: shard nodes, bucket edges, build index / S arrays
# ──────────────────────────────────────────────────────────────────────

def _balance(n, NP, src, dst, WL):
    """Permute node ids so per-(core,window) lo/hi in-edge sums are balanced.

    The lo table half = windows [0, WL) of every core (chunk-major layout).
    Returns pos[old]->new over the padded id space [0, NP*NCORES)."""
    NTOT = NP * NCORES
    W = NP // P
    half_slots = NCORES * WL * P if WL is not None else NP * (NCORES // 2)
    npad = NTOT - n
    out_deg = np.bincount(src, minlength=n)

    # phase A: split nodes into lo (cores 0..3) / hi groups with equal
    # total out-degree, so E_lo ~ E_hi.
    order = np.argsort(-out_deg, kind="stable")
    lo_set = np.zeros(n, bool)
    elo = ehi = nlo = nhi = 0
    cap_lo, cap_hi = half_slots, NTOT - half_slots - npad
    for v in order:
        if nhi >= cap_hi or (elo <= ehi and nlo < cap_lo):
            lo_set[v] = True
            elo += out_deg[v]
            nlo += 1
        else:
            ehi += out_deg[v]
            nhi += 1

    # repair: the capacity-forced tail skews group out-mass; swap pairs to
    # rebalance (each swap moves 2*(deg_u - deg_v) from lo to hi).
    imb = elo - ehi
    if abs(imb) > 8:
        big, small = (lo_set, ~lo_set) if imb > 0 else (~lo_set, lo_set)
        b_ids = np.nonzero(big)[0]
        s_ids = np.nonzero(small)[0]
        b_sorted = b_ids[np.argsort(-out_deg[b_ids], kind="stable")]
        s_sorted = s_ids[np.argsort(out_deg[s_ids], kind="stable")]
        imb = abs(imb)
        i = j = 0
        while imb > 8 and i < len(b_sorted) and j < len(s_sorted):
            u, v = b_sorted[i], s_sorted[j]
            d = int(out_deg[u]) - int(out_deg[v])
            if d <= 0:
                break
            if 2 * d > imb:
                i += 1
                continue
            lo_set[u] = not lo_set[u]
            lo_set[v] = not lo_set[v]
            imb -= 2 * d
            i += 1
            j += 1

    # phase B: greedy 2-D balancing of (lo_in, hi_in) into 128-slot buckets
    lo_in = np.bincount(dst[lo_set[src]], minlength=n).astype(np.int64)
    hi_in = np.bincount(dst[~lo_set[src]], minlength=n).astype(np.int64)
    NBK = NCORES * W
    if WL is not None:
        in_lo_b = (np.arange(NBK) % W) < WL
    else:
        in_lo_b = np.arange(NBK) < (NCORES // 2) * W
    idx_lo = np.nonzero(in_lo_b)[0]
    idx_hi = np.nonzero(~in_lo_b)[0]
    slo = np.zeros(NBK)
    shi = np.zeros(NBK)
    cnt = np.zeros(NBK, np.int64)
    bucket_of = np.empty(n, np.int64)
    order2 = np.argsort(-(lo_in + hi_in), kind="stable")
    # per-bucket ceiling: 8 blocks; the last window of each table half is
    # an overflow sink (unbounded) so unavoidable excess concentrates there
    # instead of bumping every window to 9 blocks.
    ovf = {W - 1, (WL - 1 if WL is not None else W - 1)}
    CE = np.where(np.isin(np.arange(NBK) % W, list(ovf)), 1e12, 8.0 * P)
    for v in order2:
        cand = idx_lo if lo_set[v] else idx_hi
        cand = cand[cnt[cand] < P]
        nlo_, nhi_ = slo[cand] + lo_in[v], shi[cand] + hi_in[v]
        over = (np.maximum(nlo_ - CE[cand], 0.0)
                + np.maximum(nhi_ - CE[cand], 0.0))
        score = over * 1e9 + np.maximum(nlo_, nhi_) * 1e4 + (nlo_ + nhi_)
        b = cand[np.argmin(score)]
        bucket_of[v] = b
        slo[b] += lo_in[v]
        shi[b] += hi_in[v]
        cnt[b] += 1

    # refinement: pairwise swaps to push buckets under the 8-block ceiling
    members = [[] for _ in range(NBK)]
    for v in range(n):
        members[bucket_of[v]].append(v)

    def _swap(u, v2, b, b2):
        members[b].remove(u)
        members[b2].remove(v2)
        members[b].append(v2)
        members[b2].append(u)
        bucket_of[u], bucket_of[v2] = b2, b
        dlo, dhi = lo_in[u] - lo_in[v2], hi_in[u] - hi_in[v2]
        slo[b] -= dlo
        shi[b] -= dhi
        slo[b2] += dlo
        shi[b2] += dhi

    for _ in range(8):
        over = [b for b in range(NBK) if slo[b] > CE[b] or shi[b] > CE[b]]
        if not over:
            break
        moved = False
        for b in over:
            grp = idx_lo if in_lo_b[b] else idx_hi
            guard = 0
            while (slo[b] > CE[b] or shi[b] > CE[b]) and guard < 16:
                guard += 1
                use_lo = slo[b] > CE[b]
                exc = (slo[b] if use_lo else shi[b]) - CE[b]
                done = False
                order_b2 = sorted(
                    (t for t in grp if t != b),
                    key=lambda t: (slo[t] if use_lo else shi[t]) - CE[t])
                for u in sorted(members[b],
                                key=lambda t: -(lo_in[t] if use_lo
                                                else hi_in[t]))[:32]:
                    for b2 in order_b2[:32]:
                        for v2 in members[b2]:
                            dlo = lo_in[u] - lo_in[v2]
                            dhi = hi_in[u] - hi_in[v2]
                            d_main = dlo if use_lo else dhi
                            if (d_main >= exc
                                    and slo[b] - dlo <= CE[b]
                                    and shi[b] - dhi <= CE[b]
                                    and slo[b2] + dlo <= CE[b2]
                                    and shi[b2] + dhi <= CE[b2]):
                                _swap(u, v2, b, b2)
                                done = True
                                moved = True
                                break
                        if done:
                            break
                    if done:
                        break
                if not done:
                    break
        if not moved:
            break

    # lanes within buckets; pads fill leftover (hi) slots
    ord3 = np.argsort(bucket_of, kind="stable")
    counts = np.bincount(bucket_of, minlength=NBK)
    starts = np.zeros(NBK + 1, np.int64)
    np.cumsum(counts, out=starts[1:])
    lane = np.arange(n) - starts[bucket_of[ord3]]
    b_s = bucket_of[ord3]
    c_s, w_s = b_s // W, b_s % W
    pos = np.empty(n, np.int64)
    pos[ord3] = c_s * NP + w_s * P + lane
    return pos


def _plan(NP, src, dst, real_mask):
    NTOT = NP * NCORES
    W = NP // P                                 # dst windows per core
    bounds = _ag_bounds(W)
    WL = bounds[0] if NAG == 2 else W
    # chunk-major table: rows [0, TBLH) hold windows [0, WL) of every core
    # (= AG chunk 0); both halves are contiguous and int16-indexable.
    TBLH = NCORES * WL * P if NAG == 2 else NP * (NCORES // 2)
    assert TBLH <= 32767 and NTOT - TBLH <= 32767, TBLH

    deg = np.bincount(dst, minlength=NTOT).astype(np.float32) + 1.0
    dinv = (1.0 / np.sqrt(deg)).astype(np.float32)
    dinv[~real_mask] = 0.0

    c_arr = dst // NP
    w_arr = (dst % NP) // P
    dl_arr = (dst % P).astype(np.int64)
    cs = src // NP
    ii = src % NP
    ws_arr = ii // P
    if NAG == 2:
        in_lo = ws_arr < WL
        row = np.where(
            in_lo,
            cs * (WL * P) + ii,
            TBLH + cs * ((W - WL) * P) + (ii - WL * P))
    else:
        row = src
    half = (row >= TBLH).astype(np.int64)

    order = np.lexsort((row, w_arr, c_arr, half))
    c_s, w_s, h_s = c_arr[order], w_arr[order], half[order]
    row_s, dl_s = row[order], dl_arr[order]

    key = ((h_s * NCORES + c_s) * W + w_s)
    cnt = np.bincount(key, minlength=2 * NCORES * W).reshape(2, NCORES, W)
    starts = np.zeros(2 * NCORES * W + 1, np.int64)
    np.cumsum(cnt.reshape(-1), out=starts[1:])

    LBLK = np.maximum(_cdiv(cnt[0].max(axis=0), P), 1)   # lo blocks per window
    HBLK = np.maximum(_cdiv(cnt[1].max(axis=0), P), 1)   # hi blocks per window
    lo_off = np.zeros(W + 1, np.int64)
    np.cumsum(LBLK, out=lo_off[1:])
    hi_off = np.zeros(W + 1, np.int64)
    np.cumsum(HBLK, out=hi_off[1:])
    NLO, NHI = int(lo_off[-1]), int(hi_off[-1])
    BTOT = NLO + NHI

    gidx_list, sarr_list = [], []
    for c in range(NCORES):
        fidx = np.zeros(BTOT * P, np.int16)
        fdl = np.full(BTOT * P, -1, np.int64)
        for h in range(2):
            for w in range(W):
                k = (h * NCORES + c) * W + w
                s0, s1 = starts[k], starts[k + 1]
                m = s1 - s0
                if m == 0:
                    continue
                base = (lo_off[w] if h == 0 else NLO + hi_off[w]) * P
                fidx[base:base + m] = (row_s[s0:s1]
                                       - (TBLH if h else 0)).astype(np.int16)
                fdl[base:base + m] = dl_s[s0:s1]
        gidx_list.append(np.ascontiguousarray(
            np.tile(fidx.reshape(-1, 16).T, (NCORES, 1))))
        sarr = np.zeros((P, BTOT, P), NP8)
        pos = np.nonzero(fdl >= 0)[0]
        sarr[pos % P, pos // P, fdl[pos]] = 1.0
        sarr_list.append(sarr.reshape(P, BTOT * P))

    dinv_t, valid_t = [], []
    for c in range(NCORES):
        g = c * NP + (np.arange(P)[:, None] + P * np.arange(W)[None, :])
        real = real_mask[g]
        dv = np.zeros((P, W), np.float32)
        dv[real] = dinv[g[real]]
        dinv_t.append(dv)
        valid_t.append(real.astype(np.float32))

    return dict(NP=NP, W=W, TBLH=TBLH, bounds=bounds,
                LBLK=[int(v) for v in LBLK], HBLK=[int(v) for v in HBLK],
                lo_off=[int(v) for v in lo_off], hi_off=[int(v) for v in hi_off],
                NLO=NLO, NHI=NHI, BTOT=BTOT,
                gidx=gidx_list, sarr=sarr_list,
                dinv=dinv_t, valid=valid_t)


# ──────────────────────────────────────────────────────────────────────
# device program
# ──────────────────────────────────────────────────────────────────────

def _build(ninv, NP, W, TBLH, bounds, LBLK, HBLK, lo_off, hi_off, NLO, NHI,
           BTOT, K2E, D, C, bz):
    RG = [list(range(NCORES))]
    nc = bacc.Bacc("TRN2", num_devices=NCORES, num_swdge_queues=4)

    xt_d = nc.dram_tensor("xt", [K2E, P, 2, NP], BF16, kind="ExternalInput")
    wlin_d = nc.dram_tensor("wlin", [K2E, P, 2, D], BF16, kind="ExternalInput")
    w1_d = nc.dram_tensor("w1", [1, P, 2, D], BF16, kind="ExternalInput")
    w2_d = nc.dram_tensor("w2", [1, P, 2, D], BF16, kind="ExternalInput")
    wm1_d = nc.dram_tensor("wm1", [1, P, 2, D], BF16, kind="ExternalInput")
    wm2_d = nc.dram_tensor("wm2", [1, P, 2, C], BF16, kind="ExternalInput")
    b1r_d = nc.dram_tensor("b1r", [P, D], F32, kind="ExternalInput")
    b2r_d = nc.dram_tensor("b2r", [P, D], F32, kind="ExternalInput")
    bcr_d = nc.dram_tensor("bcr", [P, C], F32, kind="ExternalInput")
    gam_d = nc.dram_tensor("gam", [P, 2], F32, kind="ExternalInput")
    bet_d = nc.dram_tensor("bet", [P, 2], F32, kind="ExternalInput")
    ident_d = nc.dram_tensor("ident", [P, P], BF16, kind="ExternalInput")
    gidx_d = nc.dram_tensor("gidx", [P, BTOT * 8], I16, kind="ExternalInput")
    sarr_d = nc.dram_tensor("sarr", [P, BTOT * P], FP8, kind="ExternalInput")
    dinv_d = nc.dram_tensor("dinv", [P, W], F32, kind="ExternalInput")
    valid_d = nc.dram_tensor("valid", [P, W], F32, kind="ExternalInput")
    out_d = nc.dram_tensor("out", [P, W, C], F32, kind="ExternalOutput")

    TDT = FP8 if FP8_TBL else BF16
    bnds = [0] + list(bounds)
    ag_in = [[nc.dram_tensor(f"ag_in{i}_{k}",
                             [bnds[k + 1] - bnds[k], P, D], TDT)
              for k in range(NAG)] for i in range(2)]
    table = [nc.dram_tensor(f"table{i}", [NCORES * NP, D], TDT,
                            addr_space="Shared") for i in range(2)]
    row_base = [0]
    for k in range(NAG):
        row_base.append(row_base[-1]
                        + NCORES * (bnds[k + 1] - bnds[k]) * P)
    DS = D + 1                                 # stats cols: C' = [h2|1]^T[h2|1]
    bn_in = nc.dram_tensor("bn_in", [P, 4], F32)
    bn_out = nc.dram_tensor("bn_out", [P, 4], F32, addr_space="Shared")

    CH = 7                                     # encoder windows per x-chunk
    starts_w = [0] + bounds[:-1]

    def ag_chunk(li):
        """collective for the next pending AG chunk of layer li's table."""
        k = ag_next[li]
        ag_next[li] += 1
        nc.gpsimd.collective_compute(
            "AllGather", AX.bypass,
            ins=[ag_in[li][k][:]],
            outs=[table[li][row_base[k]:row_base[k + 1]]],
            replica_groups=RG)

    def ag_write(li, w, src_ap):
        k = 0
        while w >= bounds[k]:
            k += 1
        nc.sync.dma_start(ag_in[li][k][w - starts_w[k]], src_ap)

    def chunk_spans(off, nblk):
        """stream block range [off, off+nblk) -> [(chunk, b0, b1)] in CB units."""
        out = []
        b = off
        while b < off + nblk:
            ci = b // CB
            b1 = min(off + nblk, (ci + 1) * CB)
            out.append((ci, b - ci * CB, b1 - ci * CB))
            b = b1
        return out

    with TileContext(nc) as tc, contextlib.ExitStack() as ctx:
        cp = ctx.enter_context(tc.tile_pool(name="const", bufs=1))
        big = ctx.enter_context(tc.tile_pool(name="big", bufs=1))
        htp = ctx.enter_context(tc.tile_pool(name="htp", bufs=2))

        nc.gpsimd.load_library(library_config.mlp)

        def cload(dram, shape, dtype, tag, src=None):
            t = cp.tile(shape, dtype, tag=tag, name=tag)
            nc.sync.dma_start(t[:], dram[:] if src is None else src)
            return t

        ident_t = cload(ident_d, [P, P], BF16, "ident")
        w1_t = cload(w1_d, [P, 2, D], BF16, "w1", src=w1_d[0])
        w2_t = cload(w2_d, [P, 2, D], BF16, "w2", src=w2_d[0])
        wm1_t = cload(wm1_d, [P, 2, D], BF16, "wm1", src=wm1_d[0])
        wm2_t = cload(wm2_d, [P, 2, C], BF16, "wm2", src=wm2_d[0])
        b1r_t = cload(b1r_d, [P, D], F32, "b1r")
        b2r_t = cload(b2r_d, [P, D], F32, "b2r")
        bcr_t = cload(bcr_d, [P, C], F32, "bcr")
        gam_t = cload(gam_d, [P, 2], F32, "gam")
        bet_t = cload(bet_d, [P, 2], F32, "bet")
        gidx_t = cload(gidx_d, [P, BTOT * 8], I16, "gidx")
        dinv_t = cload(dinv_d, [P, W], F32, "dinv")
        valid_t = cload(valid_d, [P, W], F32, "valid")

        # persistent activations: ht slots rotate h0T -> h1T -> h2T -> h4T
        ht = [htp.tile([P, 2, NP], BF16, tag="ht", name=f"ht{i}")
              for i in range(3)]
        hpre = [big.tile([P, W, D], BF16, tag=f"hpre{i}", name=f"hpre{i}")
                for i in range(2)]

        # ── encoder: h0 = x @ lin_w (node-major) → transpose → ht[0];
        # fused hpre0 = (h0 @ W1) * dinv per window + chunked AllGather
        ag_next = [0, 0]
        with tc.tile_pool(name="encw", bufs=2) as wp, \
             tc.tile_pool(name="encp", bufs=2, space="PSUM") as pp, \
             tc.tile_pool(name="xtp", bufs=2) as xtp:
            wlin_t = []
            for k2 in range(K2E):
                t = cp.tile([P, 2, D], BF16, tag=f"wlin{k2}", name=f"wlin{k2}")
                nc.sync.dma_start(t[:], wlin_d[k2])
                wlin_t.append(t)
            for wc in range(_cdiv(W, CH)):
                ws, we = wc * CH, min(W, (wc + 1) * CH)
                xtc = []
                for k2 in range(K2E):
                    t = xtp.tile([P, 2, CH * P], BF16, tag=f"xtc{k2}",
                                 name=f"xtc{k2}_{wc}")
                    nc.sync.dma_start(t[:, :, :(we - ws) * P],
                                      xt_d[k2][:, :, ws * P:we * P])
                    xtc.append(t)
                for w in range(ws, we):
                    lsl = slice((w - ws) * P, (w - ws + 1) * P)
                    sl = slice(w * P, (w + 1) * P)
                    ps = pp.tile([P, D], F32, tag="ps", name=f"eps{w}")
                    for k2 in range(K2E):
                        for r in range(2):
                            nc.tensor.matmul(
                                ps[:], xtc[k2][:, r, lsl], wlin_t[k2][:, r, :],
                                start=(k2 == 0 and r == 0),
                                stop=(k2 == K2E - 1 and r == 1))
                    hb = wp.tile([P, D], BF16, tag="hb", name=f"ehb{w}")
                    nc.vector.tensor_copy(hb[:], ps[:])
                    for r in range(2):
                        pt = pp.tile([P, P], BF16, tag="pt", name=f"ept{w}_{r}")
                        nc.tensor.transpose(pt[:], hb[:, r * P:(r + 1) * P],
                                            ident_t[:])
                        nc.vector.tensor_copy(ht[0][:, r, sl], pt[:])
                    ps2 = pp.tile([P, D], F32, tag="ps2", name=f"epre{w}")
                    for r in range(2):
                        nc.tensor.matmul(ps2[:], ht[0][:, r, sl], w1_t[:, r, :],
                                         start=(r == 0), stop=(r == 1))
                    nc.scalar.mul(hpre[0][:, w, :], ps2[:], dinv_t[:, w:w + 1])
                    if FP8_TBL:
                        af = wp.tile([P, D], FP8, tag="af", name=f"eaf{w}")
                        nc.vector.tensor_scalar(af[:], ps2[:],
                                                dinv_t[:, w:w + 1], None,
                                                op0=AX.mult)
                        ag_write(0, w, af[:])
                    else:
                        ag_write(0, w, hpre[0][:, w, :])
                    if w + 1 == bounds[ag_next[0]]:
                        ag_chunk(0)

        # ── conv layers
        qrr = [0]
        qcnt = [0, 0, 0, 0]
        qsem = [nc.alloc_semaphore(f"gq{q}") for q in range(4)]
        if ASYNC_GATHER:
            for q in range(4):
                nc.gpsimd.sem_clear(qsem[q])
        GBL = GBH = 4                           # gather-buffer depth
        NSL = 256
        nq = _cdiv(NP, NSL)
        # h3 reuses hpre[0]'s slot (dead once layer 0 finishes)
        h3 = big.tile([P, 2, NP], BF16, tag="hpre0", name="h3")
        sumps = cp.tile([P, 2, nq], F32, tag="sumps", name="sumps")
        sqps = cp.tile([P, 2, nq], F32, tag="sqps", name="sqps")
        with tc.tile_pool(name="cw", bufs=3) as wp, \
             tc.tile_pool(name="gpl", bufs=GBL) as gpl, \
             tc.tile_pool(name="gph", bufs=GBH) as gph, \
             tc.tile_pool(name="cpp", bufs=3, space="PSUM") as pp, \
             tc.tile_pool(name="cpt", bufs=2, space="PSUM") as ppt:
            for li in range(2):
                HT_out = ht[li + 1]
                br = b1r_t if li == 0 else b2r_t
                chunks = {}

                def _get_chunk(hs, ci, chunks=chunks, li=li):
                    key = (hs, ci)
                    if key in chunks:
                        return chunks[key]
                    nstream = NLO if hs == 0 else NHI
                    base_blk = (0 if hs == 0 else NLO) + ci * CB
                    nblk = min(CB, nstream - ci * CB)
                    nn = nblk * P
                    gp = gpl if hs == 0 else gph
                    gt = gp.tile([P, CB, D], TDT, tag=f"gt{hs}",
                                 name=f"gt{li}_{hs}_{ci}")
                    stt = gp.tile([P, CB, P], FP8, tag=f"st{hs}",
                                  name=f"st{li}_{hs}_{ci}")
                    tb = (table[li][0:TBLH, :] if hs == 0 else
                          table[li][TBLH:NCORES * NP, :])
                    q = qrr[0] % 4
                    if ASYNC_GATHER:
                        nc.gpsimd.dma_gather(
                            gt[:, :nblk, :], tb,
                            gidx_t[:, base_blk * 8:(base_blk + nblk) * 8],
                            nn, nn, D, single_packet=False,
                            prepare_only=True, sem=qsem[q], queue_num=q)
                        nc.gpsimd.trigger_dma(count=None, queue_num=q)
                        qcnt[q] += 16
                        # Tile's auto-gating for a prep fires at desc-gen;
                        # gate the matmul consumers on actual DMA landing.
                        nc.tensor.wait_ge(qsem[q], qcnt[q])
                    else:
                        nc.gpsimd.dma_gather(
                            gt[:, :nblk, :], tb,
                            gidx_t[:, base_blk * 8:(base_blk + nblk) * 8],
                            nn, nn, D, single_packet=False, queue_num=q)
                    qrr[0] += 1
                    nc.sync.dma_start(
                        stt[:, :nblk, :],
                        sarr_d[:, base_blk * P:(base_blk + nblk) * P])
                    chunks[key] = (gt, stt)
                    return chunks[key]

                # front-load desc-gen: queue GB lo-chunks before the first
                # hi-chunk prep (which stalls the in-order engine until the
                # hi half of the table has AllGathered)
                for ci in range(min(GBL, _cdiv(NLO, CB))):
                    _get_chunk(0, ci)
                for ci in range(min(GBH, _cdiv(NHI, CB))):
                    _get_chunk(1, ci)
                for w in range(W):
                    sl = slice(w * P, (w + 1) * P)
                    pa = pp.tile([P, D], F32, tag="ps_agg", name=f"agg{li}_{w}")
                    spans = ([(0, s) for s in chunk_spans(lo_off[w], LBLK[w])]
                             + [(1, s) for s in chunk_spans(hi_off[w], HBLK[w])])
                    nmm = sum(b1 - b0 for _, (ci, b0, b1) in spans)
                    mi = 0
                    for hs, (ci, b0, b1) in spans:
                        gt, stt = _get_chunk(hs, ci)
                        for b in range(b0, b1):
                            nc.tensor.matmul(pa[:], stt[:, b, :], gt[:, b, :],
                                             start=(mi == 0),
                                             stop=(mi == nmm - 1))
                            mi += 1
                    # self-loop: agg = pa + hpre_w (hpre carries one dinv;
                    # the epilogue scale supplies the second -> 1/deg)
                    tf = wp.tile([P, D], F32, tag="tf", name=f"tf{li}_{w}")
                    nc.vector.tensor_tensor(tf[:], pa[:], hpre[li][:, w, :],
                                            op=AX.add)
                    hb = wp.tile([P, D], BF16, tag="hb2", name=f"chb{li}_{w}")
                    if bz:
                        nc.scalar.activation(hb[:], tf[:], AFT.Relu,
                                             scale=dinv_t[:, w:w + 1])
                    else:
                        tf2 = wp.tile([P, D], F32, tag="tf2",
                                      name=f"tf2{li}_{w}")
                        nc.vector.scalar_tensor_tensor(
                            tf2[:], tf[:], dinv_t[:, w:w + 1], br[:],
                            op0=AX.mult, op1=AX.add)
                        nc.vector.tensor_scalar(hb[:], tf2[:],
                                                valid_t[:, w:w + 1],
                                                0.0, op0=AX.mult, op1=AX.max)
                    for r in range(2):
                        pt = ppt.tile([P, P], BF16, tag="pt",
                                      name=f"cpt{li}_{w}_{r}")
                        nc.tensor.transpose(pt[:], hb[:, r * P:(r + 1) * P],
                                            ident_t[:])
                        if r == 0:
                            nc.scalar.copy(HT_out[:, r, sl], pt[:])
                        else:
                            nc.vector.tensor_copy(HT_out[:, r, sl], pt[:])
                    if li == 1 and (w % 2 == 1 or w == W - 1):
                        # decoder mlp1 + BN-stat accumulation folded under
                        # the gather shadow (one 256-node slab per 2 windows)
                        q = w // 2
                        ln = min(NSL, NP - q * NSL)
                        sl2 = slice(q * NSL, q * NSL + ln)
                        for fb in range(2):
                            ps3 = pp.tile([P, NSL], F32, tag="ps_pre",
                                          name=f"ph3_{fb}_{q}")
                            for r in range(2):
                                nc.tensor.matmul(
                                    ps3[:, :ln],
                                    wm1_t[:, r, fb * P:(fb + 1) * P],
                                    HT_out[:, r, sl2],
                                    start=(r == 0), stop=(r == 1))
                            nc.vector.tensor_scalar(
                                h3[:, fb, sl2], ps3[:, :ln], 1.0, 0.0,
                                op0=AX.mult, op1=AX.add,
                                accum_out=sumps[:, fb, q:q + 1])
                            scr = wp.tile([P, NSL], BF16, tag="scr",
                                          name=f"scr{fb}_{q}")
                            nc.vector.scalar_tensor_tensor(
                                scr[:, :ln], h3[:, fb, sl2], 1.0,
                                h3[:, fb, sl2], op0=AX.mult, op1=AX.mult,
                                accum_out=sqps[:, fb, q:q + 1])
                    if li == 0:
                        ps2 = pp.tile([P, D], F32, tag="ps_pre",
                                      name=f"pre1_{w}")
                        for r in range(2):
                            nc.tensor.matmul(ps2[:], HT_out[:, r, sl],
                                             w2_t[:, r, :],
                                             start=(r == 0), stop=(r == 1))
                        nc.scalar.mul(hpre[1][:, w, :], ps2[:],
                                      dinv_t[:, w:w + 1])
                        if FP8_TBL:
                            af = wp.tile([P, D], FP8, tag="af",
                                         name=f"caf{w}")
                            nc.vector.tensor_scalar(af[:], ps2[:],
                                                    dinv_t[:, w:w + 1], None,
                                                    op0=AX.mult)
                            ag_write(1, w, af[:])
                        else:
                            ag_write(1, w, hpre[1][:, w, :])
                        if w + 1 == bounds[ag_next[1]]:
                            ag_chunk(1)

        # ── decoder: BN (stats already accumulated) + relu + mlp2 + softmax
        ht4 = htp.tile([P, 2, NP], BF16, tag="ht", name="ht4")
        with tc.tile_pool(name="dec", bufs=2) as wp, \
             tc.tile_pool(name="decp", bufs=2, space="PSUM") as pp, \
             tc.tile_pool(name="st1", bufs=1) as sp:
            sums = sp.tile([P, 2], F32, tag="sums")
            sqs = sp.tile([P, 2], F32, tag="sqs")
            for fb in range(2):
                nc.vector.reduce_sum(sums[:, fb:fb + 1], sumps[:, fb, :],
                                     axis=mybir.AxisListType.X)
                nc.vector.reduce_sum(sqs[:, fb:fb + 1], sqps[:, fb, :],
                                     axis=mybir.AxisListType.X)
            bnio = sp.tile([P, 4], F32, tag="bnio")
            nc.vector.tensor_copy(bnio[:, 0:2], sums[:])
            nc.vector.tensor_copy(bnio[:, 2:4], sqs[:])
            nc.sync.dma_start(bn_in[:], bnio[:])
            nc.gpsimd.collective_compute(
                "AllReduce", AX.add, ins=[bn_in[:]], outs=[bn_out[:]],
                replica_groups=RG)
            bns = sp.tile([P, 4], F32, tag="bns")
            nc.sync.dma_start(bns[:], bn_out[:])

            mu = sp.tile([P, 2], F32, tag="mu")
            nc.vector.tensor_scalar(mu[:], bns[:, 0:2], ninv, None, op0=AX.mult)
            msq = sp.tile([P, 2], F32, tag="msq")
            nc.vector.tensor_tensor(msq[:], mu[:], mu[:], op=AX.mult)
            var = sp.tile([P, 2], F32, tag="var")
            nc.vector.scalar_tensor_tensor(var[:], bns[:, 2:4], ninv, msq[:],
                                           op0=AX.mult, op1=AX.subtract)
            vae = sp.tile([P, 2], F32, tag="vae")
            nc.vector.tensor_scalar(vae[:], var[:], BN_EPS, None, op0=AX.add)
            sd = sp.tile([P, 2], F32, tag="sd")
            nc.scalar.activation(sd[:], vae[:], AFT.Sqrt)
            rstd = sp.tile([P, 2], F32, tag="rstd")
            nc.vector.reciprocal(rstd[:], sd[:])
            A = sp.tile([P, 2], F32, tag="A")
            nc.vector.tensor_tensor(A[:], rstd[:], gam_t[:], op=AX.mult)
            tb = sp.tile([P, 2], F32, tag="tb")
            nc.vector.tensor_tensor(tb[:], mu[:], A[:], op=AX.mult)
            B = sp.tile([P, 2], F32, tag="B")
            nc.vector.tensor_tensor(B[:], bet_t[:], tb[:], op=AX.subtract)

            # BN affine + relu: one half per engine so they run concurrently
            nc.scalar.activation(ht4[:, 0, :], h3[:, 0, :], AFT.Relu,
                                 bias=B[:, 0:1], scale=A[:, 0:1])
            t4 = sp.tile([P, NP], F32, tag="t4")
            nc.vector.tensor_scalar(t4[:], h3[:, 1, :], A[:, 1:2], B[:, 1:2],
                                    op0=AX.mult, op1=AX.add)
            nc.vector.tensor_scalar(ht4[:, 1, :], t4[:], 0.0, None,
                                    op0=AX.max)

            lg = sp.tile([P, W, C], F32, tag="lg")
            for w in range(W):
                sl = slice(w * P, (w + 1) * P)
                pl = pp.tile([P, C], F32, tag="ps_lg", name=f"plg{w}")
                for r in range(2):
                    nc.tensor.matmul(pl[:], ht4[:, r, sl], wm2_t[:, r, :],
                                     start=(r == 0), stop=(r == 1))
                nc.vector.scalar_tensor_tensor(lg[:, w, :], pl[:], 1.0, bcr_t[:],
                                               op0=AX.mult, op1=AX.add)
            ex = sp.tile([P, W, C], F32, tag="ex")
            nc.scalar.activation(ex[:].rearrange("p w c -> p (w c)"),
                                 lg[:].rearrange("p w c -> p (w c)"), AFT.Exp)
            rs = sp.tile([P, W], F32, tag="rs")
            nc.vector.reduce_sum(rs[:], ex[:], axis=mybir.AxisListType.X)
            ri = sp.tile([P, W], F32, tag="ri")
            nc.vector.reciprocal(ri[:], rs[:])
            outst = sp.tile([P, W, C], F32, tag="outst")
            for w in range(W):
                nc.vector.tensor_scalar(outst[:, w, :], ex[:, w, :],
                                        ri[:, w:w + 1], None, op0=AX.mult)
            nc.sync.dma_start(out_d[:], outst[:])

    nc.compile()
    return nc


# ──────────────────────────────────────────────────────────────────────
# NTFF profiling shim (only needed when TRACE)
# ──────────────────────────────────────────────────────────────────────

def _install_hook():
    if "antenv.axon_hooks" in sys.modules:
        return
    so_path = "/opt/axon/libaxon_pjrt.so"
    holder = {"hook": None}
    mod = types.ModuleType("antenv.axon_hooks")
    mod.set_axon_ntff_profile_hook = lambda h: holder.__setitem__("hook", h)
    mod.get_axon_ntff_profile_hook = lambda: holder["hook"]
    sys.modules["antenv.axon_hooks"] = mod
    try:
        import antenv
        antenv.axon_hooks = mod
    except ImportError:
        pass
    try:
        lib = ctypes.CDLL(so_path)
        lib.axon_start_nrt_profile.argtypes = [ctypes.POINTER(ctypes.c_int64),
                                               ctypes.c_size_t]
        lib.axon_start_nrt_profile.restype = ctypes.c_int64
        lib.axon_stop_nrt_profile.argtypes = [ctypes.c_char_p]
        lib.axon_stop_nrt_profile.restype = ctypes.c_int64

        @contextlib.contextmanager
        def _hook(output_dir, device_ids):
            import jax
            jax.devices()
            if device_ids:
                ids = (ctypes.c_int64 * len(device_ids))(*device_ids)
                rc = lib.axon_start_nrt_profile(ids, len(device_ids))
            else:
                rc = lib.axon_start_nrt_profile(None, 0)
            if rc != 0:
                raise RuntimeError(f"axon_start_nrt_profile rc={rc}")
            try:
                yield
            finally:
                nf = lib.axon_stop_nrt_profile(str(output_dir).encode())
                if nf < 0:
                    raise RuntimeError(f"axon_stop_nrt_profile rc={nf}")

        holder["hook"] = _hook
    except OSError:
        pass


# ──────────────────────────────────────────────────────────────────────
# entry point
# ──────────────────────────────────────────────────────────────────────

def kernel(x, edge_index, lin_w, conv1_w, conv1_b, conv2_w, conv2_b,
           mlp1_w, mlp1_b, bn_gamma, bn_beta, mlp2_w, mlp2_b):
    x = np.asarray(x, np.float32)
    n, g = x.shape
    D = int(np.asarray(lin_w).shape[1])
    C = int(np.asarray(mlp2_w).shape[1])
    KENC = _cdiv(g, 256) * 256
    K2E = KENC // 256

    NP = _cdiv(_cdiv(n, NCORES), P) * P
    NTOT = NP * NCORES
    ei = np.asarray(edge_index)
    src = np.asarray(ei[0], np.int64)
    dst = np.asarray(ei[1], np.int64)
    if BALANCE:
        WL = _ag_bounds(NP // P)[0] if NAG == 2 else None
        pos = _balance(n, NP, src, dst, WL)
    else:
        pos = np.arange(n, dtype=np.int64)
    inv = np.full(NTOT, -1, np.int64)
    inv[pos] = np.arange(n)
    real_mask = inv >= 0

    plan = _plan(NP, pos[src], pos[dst], real_mask)
    W, TBLH, BTOT = plan["W"], plan["TBLH"], plan["BTOT"]

    bz = (not np.any(np.asarray(conv1_b))) and (not np.any(np.asarray(conv2_b)))
    key = (n, g, D, C, NP, bz, ASYNC_GATHER, NAG, FP8_TBL,
           tuple(plan["LBLK"]), tuple(plan["HBLK"]))
    if key not in _CACHE:
        _CACHE[key] = _build(1.0 / float(n), NP, W, TBLH, plan["bounds"],
                             plan["LBLK"], plan["HBLK"],
                             plan["lo_off"], plan["hi_off"],
                             plan["NLO"], plan["NHI"], BTOT, K2E, D, C, bz)
    nc = _CACHE[key]

    shared = {
        "wlin": _pack_k(lin_w, KENC),
        "w1": _pack_k(conv1_w, D),
        "w2": _pack_k(conv2_w, D),
        "wm1": _pack_k(mlp1_w, D),
        "wm2": _pack_k(mlp2_w, D),
        "b1r": np.ascontiguousarray(
            np.broadcast_to(np.asarray(conv1_b, np.float32), (P, D))),
        "b2r": np.ascontiguousarray(
            np.broadcast_to(np.asarray(conv2_b, np.float32), (P, D))),
        "bcr": np.ascontiguousarray(
            np.broadcast_to(np.asarray(mlp2_b, np.float32), (P, C))),
        "gam": np.ascontiguousarray(
            np.asarray(bn_gamma, np.float32).reshape(2, P).T),
        "bet": np.ascontiguousarray(
            np.asarray(bn_beta, np.float32).reshape(2, P).T),
        "ident": np.eye(P, dtype=np.float32).astype(ml_dtypes.bfloat16),
    }

    in_maps = []
    for c in range(NCORES):
        loc = inv[c * NP:(c + 1) * NP]
        xs = np.zeros((NP, g), np.float32)
        sel = loc >= 0
        xs[sel] = x[loc[sel]]
        xt = _pack_k(np.ascontiguousarray(xs.T), KENC)
        in_maps.append(dict(shared,
                            xt=xt,
                            gidx=plan["gidx"][c],
                            sarr=plan["sarr"][c],
                            dinv=plan["dinv"][c],
                            valid=plan["valid"][c]))

    if TRACE:
        _install_hook()
        res = run_bass_kernel_spmd(nc, in_maps, core_ids=list(range(NCORES)),
                                   trace=True, **TRACE_KW)
        LAST["exec_time_ns"] = res.exec_time_ns
        LAST["res"] = res
    else:
        res = run_bass_kernel_spmd(nc, in_maps, core_ids=list(range(NCORES)))

    parts = []
    for c in range(NCORES):
        o = np.asarray(res.results[c]["out"])            # [P, W, C]
        parts.append(np.ascontiguousarray(o.transpose(1, 0, 2)).reshape(NP, C))
    full = np.concatenate(parts, axis=0)                 # [NTOT, C] new ids
    return full[pos].astype(np.float32)

